# revision 1
# baseline (speedup 1.0000x reference)
"""Trainium2 Bass kernel for nn_DCMSABlock (3-layer dilated causal multi-head
self-attention transformer block).

Sharding: (B=2) x (4 T-chunks of 512) across 8 cores, fully SPMD, no
collectives. Each core computes 640 tokens (512 + 128-token left halo) through
all 3 layers; attention lookback is at most 15*dil + accumulated corruption
stays below local index 105 < 128, so the last 512 tokens are exact.

Layout: residual kept transposed x^T [D=512, 640] f32 in SBUF. All matmuls
fp16 operands / fp32 PSUM. LN stats via ones-column matmuls on the tensor
engine; per-token scale rows broadcast across partitions with gpsimd
partition_broadcast. Attention computed in S^T layout (keys on partitions)
so no PE transposes are needed anywhere.
"""
import numpy as np

B, T, D, H, K, DEPTH = 2, 2048, 512, 8, 16, 3
HD = D // H          # 64
EPS = 1e-5
TT = 640             # local tokens per core (512 + 128 halo)
NT = 5               # 128-token tiles
DC = 4               # 512/128 D-chunks
P = 128
NEG = -30000.0


def _build_masks():
    """maskbias[d][k, j] for S^T tile [128 k, 256 j]; j-k = query-key distance."""
    m = np.full((DEPTH, P, 256), NEG, np.float32)
    for d in range(DEPTH):
        dil = 2 ** d
        k = np.arange(P)[:, None]
        j = np.arange(256)[None, :]
        diff = j - k
        ok = (diff >= 0) & (diff % dil == 0) & (diff < K * dil)
        m[d][ok] = 0.0
    return m.astype(np.float16)


def _trace(nonzero_bias, dbg=False, ndepth=DEPTH, reps=1):
    import concourse.bacc as bacc
    import concourse.mybir as mybir
    import concourse.tile as tile

    f16, f32 = mybir.dt.float16, mybir.dt.float32
    AF = mybir.ActivationFunctionType
    nc = bacc.Bacc(trn_type="TRN2")

    xT_in = nc.dram_tensor("xT", [D, TT], f32, kind="ExternalInput")
    wqkv_in = nc.dram_tensor("wqkv", [DEPTH, D, 3 * D], f16, kind="ExternalInput")
    wproj_in = nc.dram_tensor("wproj", [DEPTH, D, D], f16, kind="ExternalInput")
    w1_in = nc.dram_tensor("w1", [DEPTH, D, 4 * D], f16, kind="ExternalInput")
    w2_in = nc.dram_tensor("w2", [DEPTH, 4 * D, D], f16, kind="ExternalInput")
    mask_in = nc.dram_tensor("maskb", [DEPTH, P, 256], f16, kind="ExternalInput")
    ident_in = nc.dram_tensor("ident", [P, P], f16, kind="ExternalInput")
    bias_in = nc.dram_tensor("biases", [DEPTH, 4, 4 * D], f16, kind="ExternalInput")
    out_xT = nc.dram_tensor("outT", [D, TT], f32, kind="ExternalOutput")
    if dbg:
        dbg_h = nc.dram_tensor("dbg_h", [D, TT], f32, kind="ExternalOutput")
        dbg_qk = nc.dram_tensor("dbg_qk", [2 * D, TT], f32, kind="ExternalOutput")
        dbg_v = nc.dram_tensor("dbg_v", [NT * P, D], f32, kind="ExternalOutput")
        dbg_o = nc.dram_tensor("dbg_o", [D, TT], f32, kind="ExternalOutput")
        dbg_rec = nc.dram_tensor("dbg_rec", [8, TT], f32, kind="ExternalOutput")

    with tile.TileContext(nc) as tc, \
         tc.tile_pool(name="sb", bufs=1) as sb, \
         tc.tile_pool(name="tr", bufs=2) as tr, \
         tc.tile_pool(name="wq", bufs=1) as wqp, \
         tc.tile_pool(name="wres", bufs=1) as wres, \
         tc.tile_pool(name="ps", bufs=2, space="PSUM") as ps, \
         tc.tile_pool(name="psC", bufs=1, space="PSUM") as psC:

        # ---- persistent SBUF ----
        xT = [sb.tile([P, TT], f32, tag=f"xT{j}", name=f"xT{j}") for j in range(DC)]
        h16 = [sb.tile([P, TT], f16, tag=f"h{j}", name=f"h{j}") for j in range(DC)]
        qh = [sb.tile([64, TT], f16, tag=f"qh{j}", name=f"qh{j}") for j in range(8)]
        kh = [sb.tile([64, TT], f16, tag=f"kh{j}", name=f"kh{j}") for j in range(8)]
        vnat = [sb.tile([P, 2 * D], f16, tag=f"v{t}", name=f"v{t}") for t in range(NT)]
        oT = [sb.tile([P, TT], f16, tag=f"o{j}", name=f"o{j}") for j in range(DC)]
        g16 = [sb.tile([P, TT], f16, tag=f"g{m}", name=f"g{m}") for m in range(16)]
        ident = sb.tile([P, P], f16, tag="ident", name="ident")
        ones_col = sb.tile([P, 1], f16, tag="ones_c", name="ones_c")
        ones_row = sb.tile([1, TT], f16, tag="ones_r", name="ones_r")

        eps_t = sb.tile([1, 1], f32, tag="eps", name="eps")
        nc.vector.memset(eps_t[:], EPS)
        nc.vector.memset(ones_col[:], 1.0)
        nc.vector.memset(ones_row[:], 1.0)
        nc.sync.dma_start(ident[:], ident_in[:])
        maskt = [sb.tile([P, 256], f16, tag=f"mask{d}", name=f"mask{d}") for d in range(DEPTH)]
        for d in range(DEPTH):
            nc.sync.dma_start(maskt[d][:], mask_in[d])
        for j in range(DC):
            nc.sync.dma_start(xT[j][:], xT_in[128 * j:128 * (j + 1), :])
        biasr = [sb.tile([4, 4 * D], f16, tag=f"bias{d}", name=f"bias{d}") for d in range(DEPTH)]
        if any(nonzero_bias):
            for d in range(DEPTH):
                nc.sync.dma_start(biasr[d][:], bias_in[d])

        def halves(n=TT):
            return [(0, 512), (512, n)] if n > 512 else [(0, n)]

        def layernorm(dst16, ln_tag):
            """dst16[j] <- f16 normalize(xT) (scale/bias folded into weights)."""
            x16 = [tr.tile([P, TT], f16, tag=f"x16_{j}", name=f"x16_{j}", bufs=1) for j in range(DC)]
            for j in range(DC):
                nc.vector.tensor_copy(x16[j][:], xT[j][:])
            mean = ps.tile([1, TT], f32, tag="A", name="A")
            for j in range(DC):
                for lo, hi in halves():
                    nc.tensor.matmul(mean[:, lo:hi], ones_col[:], x16[j][:, lo:hi],
                                     start=(j == 0), stop=(j == DC - 1))
            mean16 = sb.tile([1, TT], f16, tag=f"m16_{ln_tag}", name=f"m16_{ln_tag}")
            nc.vector.tensor_scalar_mul(mean16[:], mean[:], 1.0 / D)
            mb = tr.tile([P, TT], f16, tag="mb", name="mb", bufs=1)
            nc.gpsimd.partition_broadcast(mb[:], mean16[:])
            s16 = [tr.tile([P, TT], f16, tag=f"s16_{j}", name=f"s16_{j}", bufs=1) for j in range(DC)]
            for j in range(DC):
                nc.gpsimd.tensor_sub(s16[j][:], x16[j][:], mb[:])
            var = ps.tile([1, TT], f32, tag="A", name="A")
            for j in range(DC):
                sq = tr.tile([P, TT], f16, tag="sq", name="sq")
                nc.vector.tensor_mul(sq[:], s16[j][:], s16[j][:])
                for lo, hi in halves():
                    nc.tensor.matmul(var[:, lo:hi], ones_col[:], sq[:, lo:hi],
                                     start=(j == 0), stop=(j == DC - 1))
            sd = sb.tile([1, TT], f32, tag=f"sd_{ln_tag}", name=f"sd_{ln_tag}")
            nc.scalar.activation(sd[:], var[:], AF.Sqrt, bias=eps_t[:], scale=1.0 / D)
            rr = sb.tile([1, TT], f32, tag=f"rr_{ln_tag}", name=f"rr_{ln_tag}")
            nc.vector.reciprocal(rr[:], sd[:])
            rr16 = sb.tile([1, TT], f16, tag=f"rr16_{ln_tag}", name=f"rr16_{ln_tag}")
            nc.vector.tensor_copy(rr16[:], rr[:])
            rb = tr.tile([P, TT], f16, tag="rb", name="rb", bufs=1)
            nc.gpsimd.partition_broadcast(rb[:], rr16[:])
            for j in range(DC):
                nc.vector.tensor_mul(dst16[j][:], s16[j][:], rb[:])

        for rep in range(reps):
          for d in range(ndepth):
            dil = 2 ** d
            # ======== LN1 ========
            layernorm(h16, f"a{d}")

            # ======== QKV ========
            wq = [wqp.tile([P, 3 * D], f16, tag=f"wqkv{c}", name=f"wqkv{c}") for c in range(DC)]
            for c in range(DC):
                nc.sync.dma_start(wq[c][:], wqkv_in[d, 128 * c:128 * (c + 1), :])
            # Q^T, K^T: weight-stationary -> [dout, t]
            for oc in range(8):
                acc = ps.tile([P, TT], f32, tag="A", name="A")
                nmm = DC + (1 if nonzero_bias[0] else 0)
                for lo, hi in halves():
                    for c in range(DC):
                        nc.tensor.matmul(acc[:, lo:hi],
                                         wq[c][:, 128 * oc:128 * (oc + 1)],
                                         h16[c][:, lo:hi],
                                         start=(c == 0), stop=(c == nmm - 1))
                    if nonzero_bias[0]:
                        nc.tensor.matmul(acc[:, lo:hi],
                                         biasr[d][0:1, 128 * oc:128 * (oc + 1)],
                                         ones_row[:, lo:hi],
                                         start=False, stop=True)
                if oc < 4:   # Q
                    nc.vector.tensor_copy(qh[2 * oc][:], acc[0:64, :])
                    nc.vector.tensor_copy(qh[2 * oc + 1][:], acc[64:128, :])
                else:        # K, folded softmax scale
                    nc.scalar.mul(kh[2 * (oc - 4)][:], acc[0:64, :], HD ** -0.5)
                    nc.scalar.mul(kh[2 * (oc - 4) + 1][:], acc[64:128, :], HD ** -0.5)
            # V: activation-stationary -> natural [t, dout]
            for t in range(NT):
                accv = ps.tile([P, D], f32, tag="B", name="B")
                nmm = DC + (1 if nonzero_bias[0] else 0)
                for c in range(DC):
                    nc.tensor.matmul(accv[:], h16[c][:, 128 * t:128 * (t + 1)],
                                     wq[c][:, 1024:1536],
                                     start=(c == 0), stop=(c == nmm - 1))
                if nonzero_bias[0]:
                    nc.tensor.matmul(accv[:], ones_row[:, 128 * t:128 * (t + 1)],
                                     biasr[d][0:1, 1024:1536],
                                     start=False, stop=True)
                nc.scalar.copy(
                    vnat[t][:].rearrange("p (h w) -> p h w", w=128)[:, :, 0:64],
                    accv[:].rearrange("p (h w) -> p h w", w=64))

            # ======== Attention ========
            for pair in range(4):
                h0, h1 = 2 * pair, 2 * pair + 1
                opr0 = ps.tile([64, TT], f32, tag="A", name="A")
                opr1 = ps.tile([64, TT], f32, tag="A", name="A")
                oprs = (opr0, opr1)
                den = psC.tile([65, TT], f32, tag="C", name="C")
                p2l = []
                for c in range(NT):
                    w = 256 if c < 4 else 128
                    s2 = ps.tile([P, 2 * w], f32, tag="B", name="B")
                    for i, h in enumerate((h0, h1)):
                        kl = kh[h][:, 128 * c:128 * (c + 1)]
                        qr = qh[h][:, 128 * c:128 * c + w]
                        nc.tensor.matmul(s2[:, w * i:w * i + w], kl, qr,
                                         start=True, stop=False)
                        nc.tensor.matmul(s2[:, w * i:w * i + w], ident[:],
                                         maskt[d][:, 0:w],
                                         start=False, stop=True)
                    p2 = tr.tile([P, 512], f16, tag="p2", name="p2")
                    nc.scalar.activation(p2[:, 0:2 * w], s2[:], AF.Exp)
                    p2l.append(p2)
                    # qtile c output: prev contribution from p2l[c-1], diag from p2l[c]
                    for i, h in enumerate((h0, h1)):
                        wp_ = 256 if c < 4 else 128
                        vl_d = vnat[c][:, 128 * h:128 * h + 64]
                        reg = slice(128 * c, 128 * (c + 1))
                        pd = p2[:, wp_ * i:wp_ * i + 128]
                        if c > 0:
                            vl_p = vnat[c - 1][:, 128 * h:128 * h + 64]
                            pp = p2l[c - 1][:, 256 * i + 128:256 * i + 256]
                            nc.tensor.matmul(oprs[i][:, reg],
                                             vl_p, pp, start=True, stop=False)
                            nc.tensor.matmul(oprs[i][:, reg],
                                             vl_d, pd, start=False, stop=True)
                            nc.tensor.matmul(den[64 * i:64 * i + 1, reg],
                                             ones_col[:], pp, start=True, stop=False)
                            nc.tensor.matmul(den[64 * i:64 * i + 1, reg],
                                             ones_col[:], pd, start=False, stop=True)
                        else:
                            nc.tensor.matmul(oprs[i][:, reg],
                                             vl_d, pd, start=True, stop=True)
                            nc.tensor.matmul(den[64 * i:64 * i + 1, reg],
                                             ones_col[:], pd, start=True, stop=True)
                reca = sb.tile([1, TT], f32, tag="reca", name="reca")
                recb = sb.tile([1, TT], f32, tag="recb", name="recb")
                nc.vector.reciprocal(reca[:], den[0:1, :])
                nc.vector.reciprocal(recb[:], den[64:65, :])
                reca16 = sb.tile([1, TT], f16, tag="reca16", name="reca16")
                recb16 = sb.tile([1, TT], f16, tag="recb16", name="recb16")
                nc.vector.tensor_copy(reca16[:], reca[:])
                nc.vector.tensor_copy(recb16[:], recb[:])
                rb2a = tr.tile([64, TT], f16, tag="rb2a", name="rb2a")
                rb2b = tr.tile([64, TT], f16, tag="rb2b", name="rb2b")
                nc.gpsimd.partition_broadcast(rb2a[:], reca16[:])
                nc.gpsimd.partition_broadcast(rb2b[:], recb16[:])
                nc.vector.tensor_mul(oT[pair][0:64, :], opr0[:], rb2a[:])
                nc.vector.tensor_mul(oT[pair][64:128, :], opr1[:], rb2b[:])
                if dbg and d == 0:
                    nc.gpsimd.dma_start(dbg_rec[2 * pair:2 * pair + 1, :], reca[:])
                    nc.gpsimd.dma_start(dbg_rec[2 * pair + 1:2 * pair + 2, :], recb[:])

            if dbg and d == 0:
                for j in range(DC):
                    nc.gpsimd.dma_start(dbg_h[128 * j:128 * (j + 1), :], h16[j][:])
                for j in range(8):
                    nc.gpsimd.dma_start(dbg_qk[64 * j:64 * (j + 1), :], qh[j][:])
                    nc.gpsimd.dma_start(dbg_qk[512 + 64 * j:512 + 64 * (j + 1), :], kh[j][:])
                for t in range(NT):
                    nc.gpsimd.dma_start(
                        dbg_v[128 * t:128 * (t + 1), :],
                        vnat[t][:].rearrange("p (h w) -> p h w", w=128)[:, :, 0:64])
                for j in range(DC):
                    nc.gpsimd.dma_start(dbg_o[128 * j:128 * (j + 1), :], oT[j][:])

            # ======== proj + residual ========
            wp = [wres.tile([P, D], f16, tag=f"wp{c}", name=f"wp{c}") for c in range(DC)]
            for c in range(DC):
                nc.sync.dma_start(wp[c][:], wproj_in[d, 128 * c:128 * (c + 1), :])
            for oc in range(DC):
                acc = ps.tile([P, TT], f32, tag="A", name="A")
                nmm = DC + (1 if nonzero_bias[1] else 0)
                for lo, hi in halves():
                    for c in range(DC):
                        nc.tensor.matmul(acc[:, lo:hi],
                                         wp[c][:, 128 * oc:128 * (oc + 1)],
                                         oT[c][:, lo:hi],
                                         start=(c == 0), stop=(c == nmm - 1))
                    if nonzero_bias[1]:
                        nc.tensor.matmul(acc[:, lo:hi],
                                         biasr[d][1:2, 128 * oc:128 * (oc + 1)],
                                         ones_row[:, lo:hi],
                                         start=False, stop=True)
                nc.vector.tensor_add(xT[oc][:], xT[oc][:], acc[:])

            # ======== LN2 ========
            layernorm(h16, f"f{d}")

            # ======== FFN ========
            ww1 = [wres.tile([P, 4 * D], f16, tag=f"ww1_{c}", name=f"ww1_{c}") for c in range(DC)]
            for c in range(DC):
                nc.sync.dma_start(ww1[c][:], w1_in[d, 128 * c:128 * (c + 1), :])
            for mc in range(16):
                acc = ps.tile([P, TT], f32, tag="A", name="A")
                nmm = DC + (1 if nonzero_bias[2] else 0)
                for lo, hi in halves():
                    for c in range(DC):
                        nc.tensor.matmul(acc[:, lo:hi],
                                         ww1[c][:, 128 * mc:128 * (mc + 1)],
                                         h16[c][:, lo:hi],
                                         start=(c == 0), stop=(c == nmm - 1))
                    if nonzero_bias[2]:
                        nc.tensor.matmul(acc[:, lo:hi],
                                         biasr[d][2:3, 128 * mc:128 * (mc + 1)],
                                         ones_row[:, lo:hi],
                                         start=False, stop=True)
                nc.scalar.activation(g16[mc][:], acc[:],
                                     AF.Identity if dbg else AF.Gelu)
            ww2 = [wres.tile([P, D], f16, tag=f"ww2_{m}", name=f"ww2_{m}") for m in range(16)]
            for m in range(16):
                nc.sync.dma_start(ww2[m][:], w2_in[d, 128 * m:128 * (m + 1), :])
            for oc in range(DC):
                acc = ps.tile([P, TT], f32, tag="A", name="A")
                nmm = 16 + (1 if nonzero_bias[3] else 0)
                for lo, hi in halves():
                    for m in range(16):
                        nc.tensor.matmul(acc[:, lo:hi],
                                         ww2[m][:, 128 * oc:128 * (oc + 1)],
                                         g16[m][:, lo:hi],
                                         start=(m == 0), stop=(m == nmm - 1))
                    if nonzero_bias[3]:
                        nc.tensor.matmul(acc[:, lo:hi],
                                         biasr[d][3:4, 128 * oc:128 * (oc + 1)],
                                         ones_row[:, lo:hi],
                                         start=False, stop=True)
                nc.vector.tensor_add(xT[oc][:], xT[oc][:], acc[:])

        for j in range(DC):
            nc.sync.dma_start(out_xT[128 * j:128 * (j + 1), :], xT[j][:])

    nc.compile()
    return nc


_CACHED = {}


def kernel(x, ln1_s, ln1_b, qkv_w, proj_w, proj_b, ln2_s, ln2_b, w1, b1, w2, b2):
    from concourse.bass_utils import run_bass_kernel_spmd

    x = np.asarray(x, np.float32)
    f = lambda a: np.asarray(a, np.float32)
    ln1_s, ln1_b, qkv_w, proj_w, proj_b = map(f, (ln1_s, ln1_b, qkv_w, proj_w, proj_b))
    ln2_s, ln2_b, w1, b1, w2, b2 = map(f, (ln2_s, ln2_b, w1, b1, w2, b2))

    # fold LN scales into following matmul weights; LN biases into bias vectors
    wqkv = (ln1_s[:, :, None] * qkv_w).astype(np.float16)
    w1e = (ln2_s[:, :, None] * w1).astype(np.float16)
    qkv_b = np.einsum('dk,dkn->dn', ln1_b, qkv_w)
    b1e = b1 + np.einsum('dk,dkn->dn', ln2_b, w1)
    biases = np.zeros((DEPTH, 4, 4 * D), np.float32)
    biases[:, 0, :3 * D] = qkv_b
    biases[:, 1, :D] = proj_b
    biases[:, 2, :] = b1e
    biases[:, 3, :D] = b2
    nonzero = (np.abs(qkv_b).max() > 0, np.abs(proj_b).max() > 0,
               np.abs(b1e).max() > 0, np.abs(b2).max() > 0)

    key = nonzero
    if key not in _CACHED:
        _CACHED[key] = _trace(nonzero)
    nc = _CACHED[key]

    shared = {
        "wqkv": wqkv,
        "wproj": proj_w.astype(np.float16),
        "w1": w1e,
        "w2": w2.astype(np.float16),
        "maskb": _build_masks(),
        "ident": np.eye(P, dtype=np.float16),
        "biases": biases.astype(np.float16),
    }
    in_maps = []
    for core in range(8):
        b, q = core // 4, core % 4
        a = max(0, 512 * q - 128)
        xs = np.ascontiguousarray(x[b, a:a + TT, :].T)  # [512, 640]
        in_maps.append({"xT": xs, **shared})

    res = run_bass_kernel_spmd(nc, in_maps, list(range(8)))

    out = np.empty((B, T, D), np.float32)
    for core in range(8):
        b, q = core // 4, core % 4
        r = res.results[core]["outT"]          # [512, 640]
        cols = r[:, 0:512] if q == 0 else r[:, 128:640]
        out[b, 512 * q:512 * (q + 1), :] = cols.T
    return out



# revision 2
# speedup vs baseline: 17.0963x; 17.0963x over previous
"""Trainium2 Bass kernel for nn_DCMSABlock (3-layer dilated causal multi-head
self-attention transformer block).

Sharding: (B=2) x (4 T-chunks of 512) across 8 cores, fully SPMD, no
collectives. Each core computes 640 tokens (512 + 128-token left halo) through
all 3 layers; attention lookback is at most 15*dil + accumulated corruption
stays below local index 105 < 128, so the last 512 tokens are exact.

Layout: residual kept transposed x^T [D=512, 640] f32 in SBUF. All matmuls
fp16 operands / fp32 PSUM. LN stats via ones-column matmuls on the tensor
engine; per-token scale rows broadcast across partitions with gpsimd
partition_broadcast. Attention computed in S^T layout (keys on partitions)
so no PE transposes are needed anywhere.

Dispatch: the jitted shard_map executable is built once and cached; weights
are content-hashed and kept device-resident (replicated across the 8 cores),
so warm calls only ship the x shards up and the output back.
"""
import hashlib
import numpy as np

B, T, D, H, K, DEPTH = 2, 2048, 512, 8, 16, 3
HD = D // H          # 64
EPS = 1e-5
TT = 640             # local tokens per core (512 + 128 halo)
NT = 5               # 128-token tiles
DC = 4               # 512/128 D-chunks
P = 128
NEG = -30000.0
NCORES = 8


def _build_masks():
    """maskbias[d][k, j] for S^T tile [128 k, 256 j]; j-k = query-key distance."""
    m = np.full((DEPTH, P, 256), NEG, np.float32)
    for d in range(DEPTH):
        dil = 2 ** d
        k = np.arange(P)[:, None]
        j = np.arange(256)[None, :]
        diff = j - k
        ok = (diff >= 0) & (diff % dil == 0) & (diff < K * dil)
        m[d][ok] = 0.0
    return m.astype(np.float16)


def _trace(nonzero_bias, dbg=False, ndepth=DEPTH, reps=1):
    import concourse.bacc as bacc
    import concourse.mybir as mybir
    import concourse.tile as tile

    f16, f32 = mybir.dt.float16, mybir.dt.float32
    AF = mybir.ActivationFunctionType
    nc = bacc.Bacc(trn_type="TRN2")

    xT_in = nc.dram_tensor("xT", [D, TT], f32, kind="ExternalInput")
    wqkv_in = nc.dram_tensor("wqkv", [DEPTH, D, 3 * D], f16, kind="ExternalInput")
    wproj_in = nc.dram_tensor("wproj", [DEPTH, D, D], f16, kind="ExternalInput")
    w1_in = nc.dram_tensor("w1", [DEPTH, D, 4 * D], f16, kind="ExternalInput")
    w2_in = nc.dram_tensor("w2", [DEPTH, 4 * D, D], f16, kind="ExternalInput")
    mask_in = nc.dram_tensor("maskb", [DEPTH, P, 256], f16, kind="ExternalInput")
    ident_in = nc.dram_tensor("ident", [P, P], f16, kind="ExternalInput")
    bias_in = nc.dram_tensor("biases", [DEPTH, 4, 4 * D], f16, kind="ExternalInput")
    out_xT = nc.dram_tensor("outT", [D, TT], f32, kind="ExternalOutput")
    if dbg:
        dbg_h = nc.dram_tensor("dbg_h", [D, TT], f32, kind="ExternalOutput")
        dbg_qk = nc.dram_tensor("dbg_qk", [2 * D, TT], f32, kind="ExternalOutput")
        dbg_v = nc.dram_tensor("dbg_v", [NT * P, D], f32, kind="ExternalOutput")
        dbg_o = nc.dram_tensor("dbg_o", [D, TT], f32, kind="ExternalOutput")
        dbg_rec = nc.dram_tensor("dbg_rec", [8, TT], f32, kind="ExternalOutput")

    with tile.TileContext(nc) as tc, \
         tc.tile_pool(name="sb", bufs=1) as sb, \
         tc.tile_pool(name="tr", bufs=2) as tr, \
         tc.tile_pool(name="wq", bufs=1) as wqp, \
         tc.tile_pool(name="wres", bufs=1) as wres, \
         tc.tile_pool(name="ps", bufs=2, space="PSUM") as ps, \
         tc.tile_pool(name="psC", bufs=1, space="PSUM") as psC:

        # ---- persistent SBUF ----
        xT = [sb.tile([P, TT], f32, tag=f"xT{j}", name=f"xT{j}") for j in range(DC)]
        h16 = [sb.tile([P, TT], f16, tag=f"h{j}", name=f"h{j}") for j in range(DC)]
        qh = [sb.tile([64, TT], f16, tag=f"qh{j}", name=f"qh{j}") for j in range(8)]
        kh = [sb.tile([64, TT], f16, tag=f"kh{j}", name=f"kh{j}") for j in range(8)]
        vnat = [sb.tile([P, 2 * D], f16, tag=f"v{t}", name=f"v{t}") for t in range(NT)]
        oT = [sb.tile([P, TT], f16, tag=f"o{j}", name=f"o{j}") for j in range(DC)]
        g16 = [sb.tile([P, TT], f16, tag=f"g{m}", name=f"g{m}") for m in range(16)]
        ident = sb.tile([P, P], f16, tag="ident", name="ident")
        ones_col = sb.tile([P, 1], f16, tag="ones_c", name="ones_c")
        ones_row = sb.tile([1, TT], f16, tag="ones_r", name="ones_r")

        eps_t = sb.tile([1, 1], f32, tag="eps", name="eps")
        nc.vector.memset(eps_t[:], EPS)
        nc.vector.memset(ones_col[:], 1.0)
        nc.vector.memset(ones_row[:], 1.0)
        nc.sync.dma_start(ident[:], ident_in[:])
        maskt = [sb.tile([P, 256], f16, tag=f"mask{d}", name=f"mask{d}") for d in range(DEPTH)]
        for d in range(DEPTH):
            nc.sync.dma_start(maskt[d][:], mask_in[d])
        for j in range(DC):
            nc.sync.dma_start(xT[j][:], xT_in[128 * j:128 * (j + 1), :])
        biasr = [sb.tile([4, 4 * D], f16, tag=f"bias{d}", name=f"bias{d}") for d in range(DEPTH)]
        if any(nonzero_bias):
            for d in range(DEPTH):
                nc.sync.dma_start(biasr[d][:], bias_in[d])

        def halves(n=TT):
            return [(0, 512), (512, n)] if n > 512 else [(0, n)]

        def layernorm(dst16, ln_tag):
            """dst16[j] <- f16 normalize(xT) (scale/bias folded into weights)."""
            x16 = [tr.tile([P, TT], f16, tag=f"x16_{j}", name=f"x16_{j}", bufs=1) for j in range(DC)]
            for j in range(DC):
                nc.vector.tensor_copy(x16[j][:], xT[j][:])
            mean = ps.tile([1, TT], f32, tag="A", name="A")
            for j in range(DC):
                for lo, hi in halves():
                    nc.tensor.matmul(mean[:, lo:hi], ones_col[:], x16[j][:, lo:hi],
                                     start=(j == 0), stop=(j == DC - 1))
            mean16 = sb.tile([1, TT], f16, tag=f"m16_{ln_tag}", name=f"m16_{ln_tag}")
            nc.vector.tensor_scalar_mul(mean16[:], mean[:], 1.0 / D)
            mb = tr.tile([P, TT], f16, tag="mb", name="mb", bufs=1)
            nc.gpsimd.partition_broadcast(mb[:], mean16[:])
            s16 = [tr.tile([P, TT], f16, tag=f"s16_{j}", name=f"s16_{j}", bufs=1) for j in range(DC)]
            for j in range(DC):
                nc.gpsimd.tensor_sub(s16[j][:], x16[j][:], mb[:])
            var = ps.tile([1, TT], f32, tag="A", name="A")
            for j in range(DC):
                sq = tr.tile([P, TT], f16, tag="sq", name="sq")
                nc.vector.tensor_mul(sq[:], s16[j][:], s16[j][:])
                for lo, hi in halves():
                    nc.tensor.matmul(var[:, lo:hi], ones_col[:], sq[:, lo:hi],
                                     start=(j == 0), stop=(j == DC - 1))
            sd = sb.tile([1, TT], f32, tag=f"sd_{ln_tag}", name=f"sd_{ln_tag}")
            nc.scalar.activation(sd[:], var[:], AF.Sqrt, bias=eps_t[:], scale=1.0 / D)
            rr = sb.tile([1, TT], f32, tag=f"rr_{ln_tag}", name=f"rr_{ln_tag}")
            nc.vector.reciprocal(rr[:], sd[:])
            rr16 = sb.tile([1, TT], f16, tag=f"rr16_{ln_tag}", name=f"rr16_{ln_tag}")
            nc.vector.tensor_copy(rr16[:], rr[:])
            rb = tr.tile([P, TT], f16, tag="rb", name="rb", bufs=1)
            nc.gpsimd.partition_broadcast(rb[:], rr16[:])
            for j in range(DC):
                nc.vector.tensor_mul(dst16[j][:], s16[j][:], rb[:])

        for rep in range(reps):
          for d in range(ndepth):
            dil = 2 ** d
            # ======== LN1 ========
            layernorm(h16, f"a{d}")

            # ======== QKV ========
            wq = [wqp.tile([P, 3 * D], f16, tag=f"wqkv{c}", name=f"wqkv{c}") for c in range(DC)]
            for c in range(DC):
                nc.sync.dma_start(wq[c][:], wqkv_in[d, 128 * c:128 * (c + 1), :])
            # Q^T, K^T: weight-stationary -> [dout, t]
            for oc in range(8):
                acc = ps.tile([P, TT], f32, tag="A", name="A")
                nmm = DC + (1 if nonzero_bias[0] else 0)
                for lo, hi in halves():
                    for c in range(DC):
                        nc.tensor.matmul(acc[:, lo:hi],
                                         wq[c][:, 128 * oc:128 * (oc + 1)],
                                         h16[c][:, lo:hi],
                                         start=(c == 0), stop=(c == nmm - 1))
                    if nonzero_bias[0]:
                        nc.tensor.matmul(acc[:, lo:hi],
                                         biasr[d][0:1, 128 * oc:128 * (oc + 1)],
                                         ones_row[:, lo:hi],
                                         start=False, stop=True)
                if oc < 4:   # Q
                    nc.vector.tensor_copy(qh[2 * oc][:], acc[0:64, :])
                    nc.vector.tensor_copy(qh[2 * oc + 1][:], acc[64:128, :])
                else:        # K, folded softmax scale
                    nc.scalar.mul(kh[2 * (oc - 4)][:], acc[0:64, :], HD ** -0.5)
                    nc.scalar.mul(kh[2 * (oc - 4) + 1][:], acc[64:128, :], HD ** -0.5)
            # V: activation-stationary -> natural [t, dout]
            for t in range(NT):
                accv = ps.tile([P, D], f32, tag="B", name="B")
                nmm = DC + (1 if nonzero_bias[0] else 0)
                for c in range(DC):
                    nc.tensor.matmul(accv[:], h16[c][:, 128 * t:128 * (t + 1)],
                                     wq[c][:, 1024:1536],
                                     start=(c == 0), stop=(c == nmm - 1))
                if nonzero_bias[0]:
                    nc.tensor.matmul(accv[:], ones_row[:, 128 * t:128 * (t + 1)],
                                     biasr[d][0:1, 1024:1536],
                                     start=False, stop=True)
                nc.scalar.copy(
                    vnat[t][:].rearrange("p (h w) -> p h w", w=128)[:, :, 0:64],
                    accv[:].rearrange("p (h w) -> p h w", w=64))

            # ======== Attention ========
            for pair in range(4):
                h0, h1 = 2 * pair, 2 * pair + 1
                opr0 = ps.tile([64, TT], f32, tag="A", name="A")
                opr1 = ps.tile([64, TT], f32, tag="A", name="A")
                oprs = (opr0, opr1)
                den = psC.tile([65, TT], f32, tag="C", name="C")
                p2l = []
                for c in range(NT):
                    w = 256 if c < 4 else 128
                    s2 = ps.tile([P, 2 * w], f32, tag="B", name="B")
                    for i, h in enumerate((h0, h1)):
                        kl = kh[h][:, 128 * c:128 * (c + 1)]
                        qr = qh[h][:, 128 * c:128 * c + w]
                        nc.tensor.matmul(s2[:, w * i:w * i + w], kl, qr,
                                         start=True, stop=False)
                        nc.tensor.matmul(s2[:, w * i:w * i + w], ident[:],
                                         maskt[d][:, 0:w],
                                         start=False, stop=True)
                    p2 = tr.tile([P, 512], f16, tag="p2", name="p2")
                    nc.scalar.activation(p2[:, 0:2 * w], s2[:], AF.Exp)
                    p2l.append(p2)
                    # qtile c output: prev contribution from p2l[c-1], diag from p2l[c]
                    for i, h in enumerate((h0, h1)):
                        wp_ = 256 if c < 4 else 128
                        vl_d = vnat[c][:, 128 * h:128 * h + 64]
                        reg = slice(128 * c, 128 * (c + 1))
                        pd = p2[:, wp_ * i:wp_ * i + 128]
                        if c > 0:
                            vl_p = vnat[c - 1][:, 128 * h:128 * h + 64]
                            pp = p2l[c - 1][:, 256 * i + 128:256 * i + 256]
                            nc.tensor.matmul(oprs[i][:, reg],
                                             vl_p, pp, start=True, stop=False)
                            nc.tensor.matmul(oprs[i][:, reg],
                                             vl_d, pd, start=False, stop=True)
                            nc.tensor.matmul(den[64 * i:64 * i + 1, reg],
                                             ones_col[:], pp, start=True, stop=False)
                            nc.tensor.matmul(den[64 * i:64 * i + 1, reg],
                                             ones_col[:], pd, start=False, stop=True)
                        else:
                            nc.tensor.matmul(oprs[i][:, reg],
                                             vl_d, pd, start=True, stop=True)
                            nc.tensor.matmul(den[64 * i:64 * i + 1, reg],
                                             ones_col[:], pd, start=True, stop=True)
                reca = sb.tile([1, TT], f32, tag="reca", name="reca")
                recb = sb.tile([1, TT], f32, tag="recb", name="recb")
                nc.vector.reciprocal(reca[:], den[0:1, :])
                nc.vector.reciprocal(recb[:], den[64:65, :])
                reca16 = sb.tile([1, TT], f16, tag="reca16", name="reca16")
                recb16 = sb.tile([1, TT], f16, tag="recb16", name="recb16")
                nc.vector.tensor_copy(reca16[:], reca[:])
                nc.vector.tensor_copy(recb16[:], recb[:])
                rb2a = tr.tile([64, TT], f16, tag="rb2a", name="rb2a")
                rb2b = tr.tile([64, TT], f16, tag="rb2b", name="rb2b")
                nc.gpsimd.partition_broadcast(rb2a[:], reca16[:])
                nc.gpsimd.partition_broadcast(rb2b[:], recb16[:])
                nc.vector.tensor_mul(oT[pair][0:64, :], opr0[:], rb2a[:])
                nc.vector.tensor_mul(oT[pair][64:128, :], opr1[:], rb2b[:])

            # ======== proj + residual ========
            wp = [wres.tile([P, D], f16, tag=f"wp{c}", name=f"wp{c}") for c in range(DC)]
            for c in range(DC):
                nc.sync.dma_start(wp[c][:], wproj_in[d, 128 * c:128 * (c + 1), :])
            for oc in range(DC):
                acc = ps.tile([P, TT], f32, tag="A", name="A")
                nmm = DC + (1 if nonzero_bias[1] else 0)
                for lo, hi in halves():
                    for c in range(DC):
                        nc.tensor.matmul(acc[:, lo:hi],
                                         wp[c][:, 128 * oc:128 * (oc + 1)],
                                         oT[c][:, lo:hi],
                                         start=(c == 0), stop=(c == nmm - 1))
                    if nonzero_bias[1]:
                        nc.tensor.matmul(acc[:, lo:hi],
                                         biasr[d][1:2, 128 * oc:128 * (oc + 1)],
                                         ones_row[:, lo:hi],
                                         start=False, stop=True)
                nc.vector.tensor_add(xT[oc][:], xT[oc][:], acc[:])

            # ======== LN2 ========
            layernorm(h16, f"f{d}")

            # ======== FFN ========
            ww1 = [wres.tile([P, 4 * D], f16, tag=f"ww1_{c}", name=f"ww1_{c}") for c in range(DC)]
            for c in range(DC):
                nc.sync.dma_start(ww1[c][:], w1_in[d, 128 * c:128 * (c + 1), :])
            for mc in range(16):
                acc = ps.tile([P, TT], f32, tag="A", name="A")
                nmm = DC + (1 if nonzero_bias[2] else 0)
                for lo, hi in halves():
                    for c in range(DC):
                        nc.tensor.matmul(acc[:, lo:hi],
                                         ww1[c][:, 128 * mc:128 * (mc + 1)],
                                         h16[c][:, lo:hi],
                                         start=(c == 0), stop=(c == nmm - 1))
                    if nonzero_bias[2]:
                        nc.tensor.matmul(acc[:, lo:hi],
                                         biasr[d][2:3, 128 * mc:128 * (mc + 1)],
                                         ones_row[:, lo:hi],
                                         start=False, stop=True)
                nc.scalar.activation(g16[mc][:], acc[:],
                                     AF.Identity if dbg else AF.Gelu)
            ww2 = [wres.tile([P, D], f16, tag=f"ww2_{m}", name=f"ww2_{m}") for m in range(16)]
            for m in range(16):
                nc.sync.dma_start(ww2[m][:], w2_in[d, 128 * m:128 * (m + 1), :])
            for oc in range(DC):
                acc = ps.tile([P, TT], f32, tag="A", name="A")
                nmm = 16 + (1 if nonzero_bias[3] else 0)
                for lo, hi in halves():
                    for m in range(16):
                        nc.tensor.matmul(acc[:, lo:hi],
                                         ww2[m][:, 128 * oc:128 * (oc + 1)],
                                         g16[m][:, lo:hi],
                                         start=(m == 0), stop=(m == nmm - 1))
                    if nonzero_bias[3]:
                        nc.tensor.matmul(acc[:, lo:hi],
                                         biasr[d][3:4, 128 * oc:128 * (oc + 1)],
                                         ones_row[:, lo:hi],
                                         start=False, stop=True)
                nc.vector.tensor_add(xT[oc][:], xT[oc][:], acc[:])

        for j in range(DC):
            nc.sync.dma_start(out_xT[128 * j:128 * (j + 1), :], xT[j][:])

    nc.compile()
    return nc


# ---------------------------------------------------------------------------
# Cached SPMD dispatch.
#
# Under axon, run_bass_kernel_spmd rebuilds a fresh jax.jit closure and
# re-uploads every input (8 replicated weight copies ~ 150 MB) on every call;
# the tunnel moves ~60 MB/s, so that dominates wall time. Here the jitted
# shard_map executable is built once per traced module and inputs are kept
# device-resident keyed by content hash: warm calls only upload tensors whose
# bytes actually changed (normally just x) and download the output.
# ---------------------------------------------------------------------------

_CACHED = {}      # nonzero_bias key -> traced nc
_EXEC = {}        # id of nc -> dispatch state


def _digest(a):
    h = hashlib.blake2b(digest_size=16)
    h.update(np.ascontiguousarray(a))
    return (a.shape, a.dtype.str, h.digest())


def _get_exec(nc, per_core_names):
    key = id(nc)
    st = _EXEC.get(key)
    if st is not None:
        return st

    import jax
    import jax.core
    import concourse.mybir as mybir
    from concourse import bass2jax
    from jax.experimental.shard_map import shard_map
    from jax.sharding import Mesh, NamedSharding, PartitionSpec

    bass2jax.install_neuronx_cc_hook()

    partition_name = nc.partition_id_tensor.name if nc.partition_id_tensor else None
    dbg_name = nc.dbg_addr.name if nc.dbg_addr is not None else None
    in_names, out_names, out_avals = [], [], []
    for alloc in nc.m.functions[0].allocations:
        if not isinstance(alloc, mybir.MemoryLocationSet):
            continue
        name = alloc.memorylocations[0].name
        if alloc.kind == "ExternalInput":
            if name != partition_name:
                in_names.append(name)
        elif alloc.kind == "ExternalOutput":
            shape = tuple(alloc.tensor_shape)
            dtype = mybir.dt.np(alloc.dtype)
            out_names.append(name)
            out_avals.append(jax.core.ShapedArray(shape, dtype))

    all_in_names = list(in_names) + list(out_names)
    if partition_name is not None:
        all_in_names.append(partition_name)

    def _body(*args):
        operands = list(args)
        if partition_name is not None:
            operands.append(bass2jax.partition_id_tensor())
        outs = bass2jax._bass_exec_p.bind(
            *operands,
            out_avals=tuple(out_avals),
            in_names=tuple(all_in_names),
            out_names=tuple(out_names),
            lowering_input_output_aliases=(),
            sim_require_finite=True,
            sim_require_nnan=True,
            nc=nc,
        )
        return tuple(outs)

    devices = jax.devices()[:NCORES]
    assert len(devices) == NCORES
    mesh = Mesh(np.asarray(devices), ("core",))
    core_spec = PartitionSpec("core")
    repl_spec = PartitionSpec()
    in_specs = tuple(
        core_spec if n in per_core_names else repl_spec for n in in_names
    ) + (core_spec,) * len(out_names)
    out_specs = (core_spec,) * len(out_names)
    fn = jax.jit(
        shard_map(_body, mesh=mesh, in_specs=in_specs,
                  out_specs=out_specs, check_rep=False),
        keep_unused=True,
    )

    core_sh = NamedSharding(mesh, core_spec)
    repl_sh = NamedSharding(mesh, repl_spec)
    zeros = [
        jax.device_put(np.zeros((NCORES * av.shape[0], *av.shape[1:]), av.dtype),
                       core_sh)
        for av in out_avals
    ]
    st = {
        "fn": fn, "in_names": in_names, "out_names": out_names,
        "out_avals": out_avals, "zeros": zeros, "core_sh": core_sh,
        "repl_sh": repl_sh, "per_core": set(per_core_names),
        "dbg_name": dbg_name, "devcache": {},
    }
    _EXEC[key] = st
    return st


def _run_cached(nc, per_core_names, host_vals):
    """host_vals: name -> np array; per-core tensors carry the global
    (NCORES*dim0, ...) concatenation, everything else the per-core value
    (replicated). Returns name -> global np output."""
    import jax

    st = _get_exec(nc, per_core_names)
    if st["dbg_name"] is not None and st["dbg_name"] not in host_vals:
        host_vals = dict(host_vals)
        host_vals[st["dbg_name"]] = np.zeros((1, 2), np.uint32)

    args = []
    for name in st["in_names"]:
        v = host_vals[name]
        dig = _digest(v)
        ent = st["devcache"].get(name)
        if ent is None or ent[0] != dig:
            sh = st["core_sh"] if name in st["per_core"] else st["repl_sh"]
            darr = jax.device_put(v, sh)
            st["devcache"][name] = (dig, darr)
            ent = st["devcache"][name]
        args.append(ent[1])

    outs = st["fn"](*args, *st["zeros"])
    return {name: np.asarray(outs[i]) for i, name in enumerate(st["out_names"])}


_WPREP = {}       # digest of raw weights -> host-prepped dict


def kernel(x, ln1_s, ln1_b, qkv_w, proj_w, proj_b, ln2_s, ln2_b, w1, b1, w2, b2):
    x = np.asarray(x, np.float32)
    f = lambda a: np.asarray(a, np.float32)
    ln1_s, ln1_b, qkv_w, proj_w, proj_b = map(f, (ln1_s, ln1_b, qkv_w, proj_w, proj_b))
    ln2_s, ln2_b, w1, b1, w2, b2 = map(f, (ln2_s, ln2_b, w1, b1, w2, b2))

    wkey = tuple(_digest(a) for a in
                 (ln1_s, ln1_b, qkv_w, proj_w, proj_b, ln2_s, ln2_b, w1, b1, w2, b2))
    prep = _WPREP.get(wkey)
    if prep is None:
        # fold LN scales into following matmul weights; LN biases into biases
        wqkv = (ln1_s[:, :, None] * qkv_w).astype(np.float16)
        w1e = (ln2_s[:, :, None] * w1).astype(np.float16)
        qkv_b = np.einsum('dk,dkn->dn', ln1_b, qkv_w)
        b1e = b1 + np.einsum('dk,dkn->dn', ln2_b, w1)
        biases = np.zeros((DEPTH, 4, 4 * D), np.float32)
        biases[:, 0, :3 * D] = qkv_b
        biases[:, 1, :D] = proj_b
        biases[:, 2, :] = b1e
        biases[:, 3, :D] = b2
        nonzero = (np.abs(qkv_b).max() > 0, np.abs(proj_b).max() > 0,
                   np.abs(b1e).max() > 0, np.abs(b2).max() > 0)
        prep = {
            "shared": {
                "wqkv": wqkv,
                "wproj": proj_w.astype(np.float16),
                "w1": w1e,
                "w2": w2.astype(np.float16),
                "maskb": _build_masks(),
                "ident": np.eye(P, dtype=np.float16),
                "biases": biases.astype(np.float16),
            },
            "nonzero": nonzero,
        }
        _WPREP[wkey] = prep

    nonzero = prep["nonzero"]
    if nonzero not in _CACHED:
        _CACHED[nonzero] = _trace(nonzero)
    nc = _CACHED[nonzero]

    # global x^T input: rows [512*core : 512*(core+1)] = core's [D, TT] slice
    xg = np.empty((NCORES * D, TT), np.float32)
    for core in range(NCORES):
        b, q = core // 4, core % 4
        a = max(0, 512 * q - 128)
        xg[D * core:D * (core + 1)] = x[b, a:a + TT, :].T

    res = _run_cached(nc, {"xT"}, {"xT": xg, **prep["shared"]})

    og = res["outT"]                        # [NCORES*D, TT]
    out = np.empty((B, T, D), np.float32)
    for core in range(NCORES):
        b, q = core // 4, core % 4
        r = og[D * core:D * (core + 1)]
        cols = r[:, 0:512] if q == 0 else r[:, 128:640]
        out[b, 512 * q:512 * (q + 1), :] = cols.T
    return out


# revision 10
# speedup vs baseline: 33.7594x; 1.9747x over previous
"""Trainium2 Bass kernel for nn_DCMSABlock (3-layer dilated causal multi-head
self-attention transformer block).

Sharding: (B=2) x (4 T-chunks of 512) across 8 cores, fully SPMD, no
collectives. Each core computes 640 tokens (512 + 128-token left halo, the
sequence-start cores zero-padded) through all 3 layers; attention lookback is
at most 15*dil + accumulated corruption stays below local index 105 < 128, so
the last 512 tokens are exact.

Layout: residual kept transposed x^T [D=512, 640] f32 in SBUF. All matmuls
fp16 operands / fp32 PSUM. LN stats via ones-column matmuls on the tensor
engine; per-token scale rows broadcast across partitions with gpsimd
partition_broadcast. Attention computed in S^T layout (keys on partitions)
so no PE transposes are needed anywhere.

Dispatch: the jitted shard_map executable is built once and cached; weights
are content-hashed and kept device-resident (replicated across the 8 cores),
so warm calls only ship the x shards up and the output back.
"""
import hashlib
import numpy as np

B, T, D, H, K, DEPTH = 2, 2048, 512, 8, 16, 3
HD = D // H          # 64
EPS = 1e-5
TT = 640             # local tokens per core (512 + 128 halo)
NT = 5               # 128-token tiles
DC = 4               # 512/128 D-chunks
P = 128
NEG = -30000.0
NCORES = 8


def _build_masks():
    """maskbias[d][k, j] for S^T tile [128 k, 256 j]; j-k = query-key distance."""
    m = np.full((DEPTH, P, 256), NEG, np.float32)
    for d in range(DEPTH):
        dil = 2 ** d
        k = np.arange(P)[:, None]
        j = np.arange(256)[None, :]
        diff = j - k
        ok = (diff >= 0) & (diff % dil == 0) & (diff < K * dil)
        m[d][ok] = 0.0
    return m.astype(np.float16)


def _trace(nonzero_bias, dbg=False, ndepth=DEPTH, reps=1):
    import concourse.bacc as bacc
    import concourse.mybir as mybir
    import concourse.tile as tile

    f16, f32 = mybir.dt.float16, mybir.dt.float32
    AF = mybir.ActivationFunctionType
    nc = bacc.Bacc(trn_type="TRN2")

    xT_in = nc.dram_tensor("xT", [D, TT], f16, kind="ExternalInput")
    wqkv_in = nc.dram_tensor("wqkv", [DEPTH, D, 3 * D], f16, kind="ExternalInput")
    wproj_in = nc.dram_tensor("wproj", [DEPTH, D, D], f16, kind="ExternalInput")
    w1_in = nc.dram_tensor("w1", [DEPTH, D, 4 * D], f16, kind="ExternalInput")
    w2_in = nc.dram_tensor("w2", [DEPTH, 4 * D, D], f16, kind="ExternalInput")
    mask_in = nc.dram_tensor("maskb", [DEPTH, P, 256], f16, kind="ExternalInput")
    mask0_in = nc.dram_tensor("maskb0", [DEPTH, P, 256], f16, kind="ExternalInput")
    ident_in = nc.dram_tensor("ident", [P, P], f16, kind="ExternalInput")
    bias_in = nc.dram_tensor("biases", [DEPTH, 4, 4 * D], f16, kind="ExternalInput")
    out_xT = nc.dram_tensor("outT", [D, 512], f16, kind="ExternalOutput")
    if dbg:
        dbg_h = nc.dram_tensor("dbg_h", [D, TT], f32, kind="ExternalOutput")
        dbg_qk = nc.dram_tensor("dbg_qk", [2 * D, TT], f32, kind="ExternalOutput")
        dbg_v = nc.dram_tensor("dbg_v", [NT * P, D], f32, kind="ExternalOutput")
        dbg_o = nc.dram_tensor("dbg_o", [D, TT], f32, kind="ExternalOutput")
        dbg_rec = nc.dram_tensor("dbg_rec", [8, TT], f32, kind="ExternalOutput")

    with tile.TileContext(nc) as tc, \
         tc.tile_pool(name="sb", bufs=1) as sb, \
         tc.tile_pool(name="tr", bufs=2) as tr, \
         tc.tile_pool(name="wq", bufs=1) as wqp, \
         tc.tile_pool(name="wres", bufs=1) as wres, \
         tc.tile_pool(name="ps", bufs=2, space="PSUM") as ps, \
         tc.tile_pool(name="psC", bufs=1, space="PSUM") as psC:

        # ---- persistent SBUF ----
        xT = [sb.tile([P, TT], f32, tag=f"xT{j}", name=f"xT{j}") for j in range(DC)]
        h16 = [sb.tile([P, TT], f16, tag=f"h{j}", name=f"h{j}") for j in range(DC)]
        qh = [sb.tile([64, TT], f16, tag=f"qh{j}", name=f"qh{j}") for j in range(8)]
        kh = [sb.tile([64, TT], f16, tag=f"kh{j}", name=f"kh{j}") for j in range(8)]
        vnat = [sb.tile([P, 2 * D], f16, tag=f"v{t}", name=f"v{t}") for t in range(NT)]
        oT = [sb.tile([P, TT], f16, tag=f"o{j}", name=f"o{j}") for j in range(DC)]
        g16 = [sb.tile([P, TT], f16, tag=f"g{m}", name=f"g{m}") for m in range(16)]
        ident = sb.tile([P, P], f16, tag="ident", name="ident")
        ones_col = sb.tile([P, 1], f16, tag="ones_c", name="ones_c")
        ones_row = sb.tile([1, TT], f16, tag="ones_r", name="ones_r")

        eps_t = sb.tile([1, 1], f32, tag="eps", name="eps")
        nc.vector.memset(eps_t[:], EPS)
        nc.vector.memset(ones_col[:], 1.0)
        nc.vector.memset(ones_row[:], 1.0)
        nc.sync.dma_start(ident[:], ident_in[:])
        maskt = [sb.tile([P, 256], f16, tag=f"mask{d}", name=f"mask{d}") for d in range(DEPTH)]
        maskt0 = [sb.tile([P, 256], f16, tag=f"mask0_{d}", name=f"mask0_{d}") for d in range(DEPTH)]
        for d in range(DEPTH):
            nc.sync.dma_start(maskt[d][:], mask_in[d])
            nc.sync.dma_start(maskt0[d][:], mask0_in[d])
        for j in range(DC):
            nc.sync.dma_start(h16[j][:], xT_in[128 * j:128 * (j + 1), :])
            nc.vector.tensor_copy(xT[j][:], h16[j][:])
        if any(nonzero_bias):
            biasr = [sb.tile([4, 4 * D], f16, tag=f"bias{d}", name=f"bias{d}") for d in range(DEPTH)]
            for d in range(DEPTH):
                nc.sync.dma_start(biasr[d][:], bias_in[d])

        def halves(n=TT):
            return [(0, 512), (512, n)] if n > 512 else [(0, n)]

        def layernorm(dst16, ln_tag):
            """dst16[j] <- f16 normalize(xT) (scale/bias folded into weights)."""
            x16 = [tr.tile([P, TT], f16, tag=f"x16_{j}", name=f"x16_{j}", bufs=1) for j in range(DC)]
            for j in range(DC):
                nc.vector.tensor_copy(x16[j][:], xT[j][:])
            mean = ps.tile([1, TT], f32, tag="A", name="A")
            for j in range(DC):
                for lo, hi in halves():
                    nc.tensor.matmul(mean[:, lo:hi], ones_col[:], x16[j][:, lo:hi],
                                     start=(j == 0), stop=(j == DC - 1))
            mean16 = sb.tile([1, TT], f16, tag=f"m16_{ln_tag}", name=f"m16_{ln_tag}")
            nc.vector.tensor_scalar_mul(mean16[:], mean[:], 1.0 / D)
            mb = tr.tile([P, TT], f16, tag="mb", name="mb", bufs=1)
            nc.gpsimd.partition_broadcast(mb[:], mean16[:])
            s16 = [tr.tile([P, TT], f16, tag=f"s16_{j}", name=f"s16_{j}", bufs=1) for j in range(DC)]
            for j in range(DC):
                nc.gpsimd.tensor_sub(s16[j][:], x16[j][:], mb[:])
            var = ps.tile([1, TT], f32, tag="A", name="A")
            for j in range(DC):
                sq = tr.tile([P, TT], f16, tag="sq", name="sq")
                nc.vector.tensor_mul(sq[:], s16[j][:], s16[j][:])
                for lo, hi in halves():
                    nc.tensor.matmul(var[:, lo:hi], ones_col[:], sq[:, lo:hi],
                                     start=(j == 0), stop=(j == DC - 1))
            sd = sb.tile([1, TT], f32, tag=f"sd_{ln_tag}", name=f"sd_{ln_tag}")
            nc.scalar.activation(sd[:], var[:], AF.Sqrt, bias=eps_t[:], scale=1.0 / D)
            rr = sb.tile([1, TT], f32, tag=f"rr_{ln_tag}", name=f"rr_{ln_tag}")
            nc.vector.reciprocal(rr[:], sd[:])
            rr16 = sb.tile([1, TT], f16, tag=f"rr16_{ln_tag}", name=f"rr16_{ln_tag}")
            nc.vector.tensor_copy(rr16[:], rr[:])
            rb = tr.tile([P, TT], f16, tag="rb", name="rb", bufs=1)
            nc.gpsimd.partition_broadcast(rb[:], rr16[:])
            for j in range(DC):
                nc.vector.tensor_mul(dst16[j][:], s16[j][:], rb[:])

        for rep in range(reps):
          for d in range(ndepth):
            dil = 2 ** d
            # ======== LN1 ========
            layernorm(h16, f"a{d}")

            # ======== QKV ========
            wq = [wqp.tile([P, 3 * D], f16, tag=f"wqkv{c}", name=f"wqkv{c}") for c in range(DC)]
            for c in range(DC):
                nc.sync.dma_start(wq[c][:], wqkv_in[d, 128 * c:128 * (c + 1), :])
            # Q^T, K^T: weight-stationary -> [dout, t]
            for oc in range(8):
                acc = ps.tile([P, TT], f32, tag="A", name="A")
                nmm = DC + (1 if nonzero_bias[0] else 0)
                for lo, hi in halves():
                    for c in range(DC):
                        nc.tensor.matmul(acc[:, lo:hi],
                                         wq[c][:, 128 * oc:128 * (oc + 1)],
                                         h16[c][:, lo:hi],
                                         start=(c == 0), stop=(c == nmm - 1))
                    if nonzero_bias[0]:
                        nc.tensor.matmul(acc[:, lo:hi],
                                         biasr[d][0:1, 128 * oc:128 * (oc + 1)],
                                         ones_row[:, lo:hi],
                                         start=False, stop=True)
                if oc < 4:   # Q
                    nc.vector.tensor_copy(qh[2 * oc][:], acc[0:64, :])
                    nc.vector.tensor_copy(qh[2 * oc + 1][:], acc[64:128, :])
                else:        # K, folded softmax scale
                    nc.scalar.mul(kh[2 * (oc - 4)][:], acc[0:64, :], HD ** -0.5)
                    nc.scalar.mul(kh[2 * (oc - 4) + 1][:], acc[64:128, :], HD ** -0.5)
            # V: activation-stationary -> natural [t, dout]
            for t in range(NT):
                accv = ps.tile([P, D], f32, tag="B", name="B")
                nmm = DC + (1 if nonzero_bias[0] else 0)
                for c in range(DC):
                    nc.tensor.matmul(accv[:], h16[c][:, 128 * t:128 * (t + 1)],
                                     wq[c][:, 1024:1536],
                                     start=(c == 0), stop=(c == nmm - 1))
                if nonzero_bias[0]:
                    nc.tensor.matmul(accv[:], ones_row[:, 128 * t:128 * (t + 1)],
                                     biasr[d][0:1, 1024:1536],
                                     start=False, stop=True)
                nc.scalar.copy(
                    vnat[t][:].rearrange("p (h w) -> p h w", w=128)[:, :, 0:64],
                    accv[:].rearrange("p (h w) -> p h w", w=64))

            # ======== Attention ========
            for pair in range(4):
                h0, h1 = 2 * pair, 2 * pair + 1
                opr0 = ps.tile([64, TT], f32, tag="A", name="A")
                opr1 = ps.tile([64, TT], f32, tag="A", name="A")
                oprs = (opr0, opr1)
                den = psC.tile([65, TT], f32, tag="C", name="C")
                p2l = []
                for c in range(NT):
                    w = 256 if c < 4 else 128
                    s2 = ps.tile([P, 2 * w], f32, tag="B", name="B")
                    for i, h in enumerate((h0, h1)):
                        kl = kh[h][:, 128 * c:128 * (c + 1)]
                        qr = qh[h][:, 128 * c:128 * c + w]
                        nc.tensor.matmul(s2[:, w * i:w * i + w], kl, qr,
                                         start=True, stop=False)
                        mt = maskt0[d] if c == 0 else maskt[d]
                        nc.tensor.matmul(s2[:, w * i:w * i + w], ident[:],
                                         mt[:, 0:w],
                                         start=False, stop=True)
                    p2 = tr.tile([P, 512], f16, tag="p2", name="p2")
                    nc.scalar.activation(p2[:, 0:2 * w], s2[:], AF.Exp)
                    p2l.append(p2)
                    # qtile c output: prev contribution from p2l[c-1], diag from p2l[c]
                    for i, h in enumerate((h0, h1)):
                        wp_ = 256 if c < 4 else 128
                        vl_d = vnat[c][:, 128 * h:128 * h + 64]
                        reg = slice(128 * c, 128 * (c + 1))
                        pd = p2[:, wp_ * i:wp_ * i + 128]
                        if c > 0:
                            vl_p = vnat[c - 1][:, 128 * h:128 * h + 64]
                            pp = p2l[c - 1][:, 256 * i + 128:256 * i + 256]
                            nc.tensor.matmul(oprs[i][:, reg],
                                             vl_p, pp, start=True, stop=False)
                            nc.tensor.matmul(oprs[i][:, reg],
                                             vl_d, pd, start=False, stop=True)
                            nc.tensor.matmul(den[64 * i:64 * i + 1, reg],
                                             ones_col[:], pp, start=True, stop=False)
                            nc.tensor.matmul(den[64 * i:64 * i + 1, reg],
                                             ones_col[:], pd, start=False, stop=True)
                        else:
                            nc.tensor.matmul(oprs[i][:, reg],
                                             vl_d, pd, start=True, stop=True)
                            nc.tensor.matmul(den[64 * i:64 * i + 1, reg],
                                             ones_col[:], pd, start=True, stop=True)
                reca = sb.tile([1, TT], f32, tag="reca", name="reca")
                recb = sb.tile([1, TT], f32, tag="recb", name="recb")
                nc.vector.reciprocal(reca[:], den[0:1, :])
                nc.vector.reciprocal(recb[:], den[64:65, :])
                reca16 = sb.tile([1, TT], f16, tag="reca16", name="reca16")
                recb16 = sb.tile([1, TT], f16, tag="recb16", name="recb16")
                nc.vector.tensor_copy(reca16[:], reca[:])
                nc.vector.tensor_copy(recb16[:], recb[:])
                rb2a = tr.tile([64, TT], f16, tag="rb2a", name="rb2a")
                rb2b = tr.tile([64, TT], f16, tag="rb2b", name="rb2b")
                nc.gpsimd.partition_broadcast(rb2a[:], reca16[:])
                nc.gpsimd.partition_broadcast(rb2b[:], recb16[:])
                nc.vector.tensor_mul(oT[pair][0:64, :], opr0[:], rb2a[:])
                nc.vector.tensor_mul(oT[pair][64:128, :], opr1[:], rb2b[:])

            # ======== proj + residual ========
            wp = [wres.tile([P, D], f16, tag=f"wp{c}", name=f"wp{c}") for c in range(DC)]
            for c in range(DC):
                nc.sync.dma_start(wp[c][:], wproj_in[d, 128 * c:128 * (c + 1), :])
            for oc in range(DC):
                acc = ps.tile([P, TT], f32, tag="A", name="A")
                nmm = DC + (1 if nonzero_bias[1] else 0)
                for lo, hi in halves():
                    for c in range(DC):
                        nc.tensor.matmul(acc[:, lo:hi],
                                         wp[c][:, 128 * oc:128 * (oc + 1)],
                                         oT[c][:, lo:hi],
                                         start=(c == 0), stop=(c == nmm - 1))
                    if nonzero_bias[1]:
                        nc.tensor.matmul(acc[:, lo:hi],
                                         biasr[d][1:2, 128 * oc:128 * (oc + 1)],
                                         ones_row[:, lo:hi],
                                         start=False, stop=True)
                nc.vector.tensor_add(xT[oc][:], xT[oc][:], acc[:])

            # ======== LN2 ========
            layernorm(h16, f"f{d}")

            # ======== FFN ========
            ww1 = [wres.tile([P, 4 * D], f16, tag=f"ww1_{c}", name=f"ww1_{c}") for c in range(DC)]
            for c in range(DC):
                nc.sync.dma_start(ww1[c][:], w1_in[d, 128 * c:128 * (c + 1), :])
            for mc in range(16):
                acc = ps.tile([P, TT], f32, tag="A", name="A")
                nmm = DC + (1 if nonzero_bias[2] else 0)
                for lo, hi in halves():
                    for c in range(DC):
                        nc.tensor.matmul(acc[:, lo:hi],
                                         ww1[c][:, 128 * mc:128 * (mc + 1)],
                                         h16[c][:, lo:hi],
                                         start=(c == 0), stop=(c == nmm - 1))
                    if nonzero_bias[2]:
                        nc.tensor.matmul(acc[:, lo:hi],
                                         biasr[d][2:3, 128 * mc:128 * (mc + 1)],
                                         ones_row[:, lo:hi],
                                         start=False, stop=True)
                nc.scalar.activation(g16[mc][:], acc[:],
                                     AF.Identity if dbg else AF.Gelu)
            ww2 = [wres.tile([P, D], f16, tag=f"ww2_{m}", name=f"ww2_{m}") for m in range(16)]
            for m in range(16):
                nc.sync.dma_start(ww2[m][:], w2_in[d, 128 * m:128 * (m + 1), :])
            for oc in range(DC):
                acc = ps.tile([P, TT], f32, tag="A", name="A")
                nmm = 16 + (1 if nonzero_bias[3] else 0)
                for lo, hi in halves():
                    for m in range(16):
                        nc.tensor.matmul(acc[:, lo:hi],
                                         ww2[m][:, 128 * oc:128 * (oc + 1)],
                                         g16[m][:, lo:hi],
                                         start=(m == 0), stop=(m == nmm - 1))
                    if nonzero_bias[3]:
                        nc.tensor.matmul(acc[:, lo:hi],
                                         biasr[d][3:4, 128 * oc:128 * (oc + 1)],
                                         ones_row[:, lo:hi],
                                         start=False, stop=True)
                nc.vector.tensor_add(xT[oc][:], xT[oc][:], acc[:])

        for j in range(DC):
            nc.vector.tensor_copy(h16[j][:, 0:512], xT[j][:, 128:640])
            nc.sync.dma_start(out_xT[128 * j:128 * (j + 1), :], h16[j][:, 0:512])

    nc.compile()
    return nc


# ---------------------------------------------------------------------------
# Cached SPMD dispatch.
#
# Under axon, run_bass_kernel_spmd rebuilds a fresh jax.jit closure and
# re-uploads every input (8 replicated weight copies ~ 150 MB) on every call;
# the tunnel moves ~60 MB/s, so that dominates wall time. Here the executable
# is AOT-compiled once per traced module (with bass_effect suppressed for C++
# fast-path dispatch) and inputs are kept device-resident keyed by content
# hash: warm calls only upload tensors whose bytes actually changed (normally
# nothing, or just x) and download the f16 output (~4 MB).
#
# All 8 cores see a uniform 640-token window [512q-128, 512q+512); the q==0
# cores get 128 zero tokens prepended, and a per-core mask for the first key
# tile (maskb0) keeps those padding tokens out of real queries' softmax
# (padding queries self-attend only, so their denominator stays 1). Every
# core then emits the same output window, local columns [128, 640).
# ---------------------------------------------------------------------------

_CACHED = {}      # nonzero_bias key -> traced nc
_EXEC = {}        # id of nc -> dispatch state
_IDMEMO = {}      # id(arr) -> (arr, digest); strong ref pins the id


def _digest(a):
    h = hashlib.blake2b(digest_size=16)
    h.update(np.ascontiguousarray(a))
    return (a.shape, a.dtype.str, h.digest())


def _digest_memo(a):
    e = _IDMEMO.get(id(a))
    if e is not None and e[0] is a:
        return e[1]
    d = _digest(a)
    _IDMEMO[id(a)] = (a, d)
    return d


def _get_exec(nc, per_core_names, sample_vals):
    """Build (once) the AOT-compiled shard_map executable for nc.

    sample_vals: name -> host np array with the global shape (per-core
    tensors carry the (NCORES*dim0, ...) concatenation).
    """
    key = id(nc)
    st = _EXEC.get(key)
    if st is not None:
        return st

    import jax
    import jax.core
    import concourse.mybir as mybir
    from concourse import bass2jax
    from jax.experimental.shard_map import shard_map
    from jax.sharding import Mesh, NamedSharding, PartitionSpec

    bass2jax.install_neuronx_cc_hook()

    partition_name = nc.partition_id_tensor.name if nc.partition_id_tensor else None
    dbg_name = nc.dbg_addr.name if nc.dbg_addr is not None else None
    in_names, out_names, out_avals = [], [], []
    for alloc in nc.m.functions[0].allocations:
        if not isinstance(alloc, mybir.MemoryLocationSet):
            continue
        name = alloc.memorylocations[0].name
        if alloc.kind == "ExternalInput":
            if name != partition_name:
                in_names.append(name)
        elif alloc.kind == "ExternalOutput":
            shape = tuple(alloc.tensor_shape)
            dtype = mybir.dt.np(alloc.dtype)
            out_names.append(name)
            out_avals.append(jax.core.ShapedArray(shape, dtype))

    all_in_names = list(in_names) + list(out_names)
    if partition_name is not None:
        all_in_names.append(partition_name)

    def _body(*args):
        operands = list(args)
        if partition_name is not None:
            operands.append(bass2jax.partition_id_tensor())
        outs = bass2jax._bass_exec_p.bind(
            *operands,
            out_avals=tuple(out_avals),
            in_names=tuple(all_in_names),
            out_names=tuple(out_names),
            lowering_input_output_aliases=(),
            sim_require_finite=True,
            sim_require_nnan=True,
            nc=nc,
        )
        return tuple(outs)

    devices = jax.devices()[:NCORES]
    assert len(devices) == NCORES
    mesh = Mesh(np.asarray(devices), ("core",))
    core_spec = PartitionSpec("core")
    repl_spec = PartitionSpec()
    in_specs = tuple(
        core_spec if n in per_core_names else repl_spec for n in in_names
    ) + (core_spec,) * len(out_names)
    out_specs = (core_spec,) * len(out_names)
    sm = shard_map(_body, mesh=mesh, in_specs=in_specs,
                   out_specs=out_specs, check_rep=False)

    core_sh = NamedSharding(mesh, core_spec)
    repl_sh = NamedSharding(mesh, repl_spec)
    zeros = [
        jax.device_put(np.zeros((NCORES * av.shape[0], *av.shape[1:]), av.dtype),
                       core_sh)
        for av in out_avals
    ]

    devcache = {}
    dev_args = []
    for name in in_names:
        if name == dbg_name and name not in sample_vals:
            v = np.zeros((1, 2), np.uint32)
        else:
            v = sample_vals[name]
        sh = core_sh if name in per_core_names else repl_sh
        darr = jax.device_put(v, sh)
        devcache[name] = (_digest_memo(v), darr)
        dev_args.append(darr)

    fn = bass2jax.fast_dispatch_compile(
        lambda: jax.jit(sm, keep_unused=True).lower(*dev_args, *zeros).compile()
    )

    st = {
        "fn": fn, "in_names": in_names, "out_names": out_names,
        "zeros": zeros, "core_sh": core_sh, "repl_sh": repl_sh,
        "per_core": set(per_core_names), "dbg_name": dbg_name,
        "devcache": devcache,
    }
    _EXEC[key] = st
    return st


_WPREP = {}       # digest of raw weights -> host-prepped dict
_MASK0_Q0 = None


def _mask0_q0():
    """First-key-tile mask for sequence-start cores: padding queries (j<128)
    self-attend only; real queries (j>=128) see no padding keys."""
    global _MASK0_Q0
    if _MASK0_Q0 is None:
        m = np.full((P, 256), NEG, np.float32)
        np.fill_diagonal(m[:, :P], 0.0)
        _MASK0_Q0 = np.broadcast_to(m, (DEPTH, P, 256)).astype(np.float16)
    return _MASK0_Q0


def _build_xg(x):
    """Global x^T f16 input: rows [512c, 512(c+1)) = core c's [D, TT] window
    of tokens [512q-128, 512q+512), zero-padded at the sequence start."""
    xg = np.zeros((NCORES * D, TT), np.float16)
    for core in range(NCORES):
        b, q = core // 4, core % 4
        if q == 0:
            xg[D * core:D * (core + 1), 128:] = x[b, 0:512, :].T
        else:
            xg[D * core:D * (core + 1)] = x[b, 512 * q - 128:512 * q + 512, :].T
    return xg


def kernel(x, ln1_s, ln1_b, qkv_w, proj_w, proj_b, ln2_s, ln2_b, w1, b1, w2, b2):
    import jax

    x = np.asarray(x, np.float32)
    f = lambda a: np.asarray(a, np.float32)
    ln1_s, ln1_b, qkv_w, proj_w, proj_b = map(f, (ln1_s, ln1_b, qkv_w, proj_w, proj_b))
    ln2_s, ln2_b, w1, b1, w2, b2 = map(f, (ln2_s, ln2_b, w1, b1, w2, b2))

    wkey = tuple(_digest_memo(a) for a in
                 (ln1_s, ln1_b, qkv_w, proj_w, proj_b, ln2_s, ln2_b, w1, b1, w2, b2))
    prep = _WPREP.get(wkey)
    if prep is None:
        # fold LN scales into following matmul weights; LN biases into biases
        wqkv = (ln1_s[:, :, None] * qkv_w).astype(np.float16)
        w1e = (ln2_s[:, :, None] * w1).astype(np.float16)
        qkv_b = np.einsum('dk,dkn->dn', ln1_b, qkv_w)
        b1e = b1 + np.einsum('dk,dkn->dn', ln2_b, w1)
        biases = np.zeros((DEPTH, 4, 4 * D), np.float32)
        biases[:, 0, :3 * D] = qkv_b
        biases[:, 1, :D] = proj_b
        biases[:, 2, :] = b1e
        biases[:, 3, :D] = b2
        nonzero = (np.abs(qkv_b).max() > 0, np.abs(proj_b).max() > 0,
                   np.abs(b1e).max() > 0, np.abs(b2).max() > 0)
        masks = _build_masks()
        mask0 = np.empty((NCORES * DEPTH, P, 256), np.float16)
        for core in range(NCORES):
            mask0[DEPTH * core:DEPTH * (core + 1)] = (
                _mask0_q0() if core % 4 == 0 else masks)
        prep = {
            "shared": {
                "wqkv": wqkv,
                "wproj": proj_w.astype(np.float16),
                "w1": w1e,
                "w2": w2.astype(np.float16),
                "maskb": masks,
                "maskb0": mask0,
                "ident": np.eye(P, dtype=np.float16),
                "biases": biases.astype(np.float16),
            },
            "nonzero": nonzero,
        }
        _WPREP[wkey] = prep

    nonzero = prep["nonzero"]
    if nonzero not in _CACHED:
        _CACHED[nonzero] = _trace(nonzero)
    nc = _CACHED[nonzero]

    xdig = _digest_memo(x)
    st = _EXEC.get(id(nc))
    if st is None:
        st = _get_exec(nc, {"xT", "maskb0"},
                       {"xT": _build_xg(x), **prep["shared"]})
        st["devcache"]["xT"] = (xdig, st["devcache"]["xT"][1])

    args = []
    for name in st["in_names"]:
        if name == "xT":
            ent = st["devcache"]["xT"]
            if ent[0] != xdig:
                darr = jax.device_put(_build_xg(x), st["core_sh"])
                ent = (xdig, darr)
                st["devcache"]["xT"] = ent
            args.append(ent[1])
            continue
        v = (np.zeros((1, 2), np.uint32) if name == st["dbg_name"]
             and name not in prep["shared"] else prep["shared"][name])
        dig = _digest_memo(v)
        ent = st["devcache"].get(name)
        if ent is None or ent[0] != dig:
            sh = st["core_sh"] if name in st["per_core"] else st["repl_sh"]
            ent = (dig, jax.device_put(v, sh))
            st["devcache"][name] = ent
        args.append(ent[1])

    outs = st["fn"](*args, *st["zeros"])
    og = np.asarray(outs[st["out_names"].index("outT")])   # [NCORES*D, 512] f16

    out = np.empty((B, T, D), np.float32)
    for core in range(NCORES):
        b, q = core // 4, core % 4
        out[b, 512 * q:512 * (q + 1), :] = og[D * core:D * (core + 1)].T
    return out
